# revision 1
# baseline (speedup 1.0000x reference)
"""Reverse-time forget-mult recurrence on 8 Trainium2 NeuronCores.

h_t = f_t*x_t + (1-f_t)*h_{t+1}, h_{T+1}=0, over [T=2048, B=16, D=1024].

Strategy: shard D across the 8 cores (128 channels each) — the recurrence is
elementwise over (B, D), sequential only in T, so no cross-core communication.
On the host, each core's shard is laid out as [B*D_shard, T] = [2048, 2048]
with the T axis reversed, so on-device the whole problem is, per 128-row
block: two contiguous 1 MB DMAs in, a = 1-f (Scalar engine), g = f*x
(Vector engine), one hardware tensor_tensor_scan over the full T in a single
instruction (initial state 0), and one contiguous 1 MB DMA out. The kernel is
memory-bound: 48 MB of HBM traffic per core.
"""

import numpy as np

T, B, D = 2048, 16, 1024
NCORES = 8
DS = D // NCORES          # 128 channels per core
ROWS = B * DS             # 2048 recurrence lanes per core
PB = 128                  # partitions per block
NBLK = ROWS // PB

_cached = {}


def _build():
    import concourse.bacc as bacc
    import concourse.mybir as mybir
    import concourse.tile as tile

    nc = bacc.Bacc("TRN2", target_bir_lowering=False, debug=False, num_devices=NCORES)
    f_in = nc.dram_tensor("f_in", [ROWS, T], mybir.dt.float32, kind="ExternalInput").ap()
    x_in = nc.dram_tensor("x_in", [ROWS, T], mybir.dt.float32, kind="ExternalInput").ap()
    h_out = nc.dram_tensor("h_out", [ROWS, T], mybir.dt.float32, kind="ExternalOutput").ap()

    with tile.TileContext(nc) as tc:
        with tc.tile_pool(name="io", bufs=3) as io_pool, tc.tile_pool(name="tmp", bufs=3) as tmp_pool:
            for r in range(NBLK):
                sl = slice(PB * r, PB * (r + 1))
                f_t = io_pool.tile([PB, T], mybir.dt.float32, tag="f")
                nc.sync.dma_start(out=f_t[:], in_=f_in[sl, :])
                x_t = io_pool.tile([PB, T], mybir.dt.float32, tag="x")
                nc.sync.dma_start(out=x_t[:], in_=x_in[sl, :])
                a_t = tmp_pool.tile([PB, T], mybir.dt.float32, tag="a")
                nc.scalar.activation(
                    a_t[:], f_t[:], mybir.ActivationFunctionType.Copy, bias=1.0, scale=-1.0
                )
                g_t = tmp_pool.tile([PB, T], mybir.dt.float32, tag="g")
                nc.vector.tensor_mul(g_t[:], f_t[:], x_t[:])
                h_t = tmp_pool.tile([PB, T], mybir.dt.float32, tag="h")
                nc.vector.tensor_tensor_scan(
                    h_t[:], a_t[:], g_t[:], 0.0, mybir.AluOpType.mult, mybir.AluOpType.add
                )
                nc.sync.dma_start(out=h_out[sl, :], in_=h_t[:])
    nc.compile()
    return nc


def _get_nc():
    if "nc" not in _cached:
        _cached["nc"] = _build()
    return _cached["nc"]


def _shard(arr):
    """[T, B, D] -> per-core [B*DS, T] with T reversed (device scans forward)."""
    # view as [B, D, T] with T reversed: v[b, d, t] = arr[T-1-t, b, d]
    v = arr[::-1].transpose(1, 2, 0)  # strided view, no copy
    return [
        np.ascontiguousarray(v[:, DS * c : DS * (c + 1), :]).reshape(ROWS, T)
        for c in range(NCORES)
    ]


def _run(f, x, trace=False):
    from concourse.bass_utils import run_bass_kernel_spmd

    f = np.asarray(f, dtype=np.float32)
    x = np.asarray(x, dtype=np.float32)
    assert f.shape == (T, B, D) and x.shape == (T, B, D)

    nc = _get_nc()
    f_shards = _shard(f)
    x_shards = _shard(x)
    in_maps = [{"f_in": f_shards[c], "x_in": x_shards[c]} for c in range(NCORES)]
    res = run_bass_kernel_spmd(nc, in_maps, core_ids=list(range(NCORES)), trace=trace)

    out = np.empty((B, D, T), dtype=np.float32)
    for c in range(NCORES):
        out[:, DS * c : DS * (c + 1), :] = (
            res.results[c]["h_out"].reshape(B, DS, T)[:, :, ::-1]
        )
    full = np.ascontiguousarray(out.transpose(2, 0, 1)).reshape(T * B, D)
    return full, res


def kernel(f, x):
    return _run(f, x, trace=False)[0]


# revision 2
# speedup vs baseline: 1.1402x; 1.1402x over previous
"""Reverse-time forget-mult recurrence on 8 Trainium2 NeuronCores.

h_t = f_t*x_t + (1-f_t)*h_{t+1}, h_{T+1}=0, over [T=2048, B=16, D=1024].

Strategy: shard D across the 8 cores (128 channels each) — the recurrence is
elementwise over (B, D), sequential only in T, so no cross-core communication.
On the host, each core's shard is laid out partition-major as [D_shard=128,
B=16, T] with the T axis reversed, so each (d, b) lane's full time series is
contiguous and the device scans forward. Per 2-block step the device does one
contiguous 2 MB DMA per tensor (16 KB per-partition lines), computes
a = 1-f on the Scalar engine and g = f*x on GpSimd (keeping the Vector
engine free), and runs the whole recurrence for 128 lanes x 2048 steps in a
single hardware tensor_tensor_scan instruction (initial state 0) on Vector.
Loads issue on the Sync HWDGE ring, stores on the Scalar ring, so writes
don't head-of-line-block reads. The kernel is memory-bound: 48 MB of HBM
traffic per core.
"""

import numpy as np

T, B, D = 2048, 16, 1024
NCORES = 8
DS = D // NCORES          # 128 channels per core -> the SBUF partition dim
NBLK = B                  # 16 blocks of [128, T] per core
RB = 2                    # row-blocks per DMA (2 MB transfers)
PB = 128

_cached = {}


def _build():
    import concourse.bacc as bacc
    import concourse.mybir as mybir
    import concourse.tile as tile

    f32 = mybir.dt.float32
    nc = bacc.Bacc("TRN2", target_bir_lowering=False, debug=False, num_devices=NCORES)
    f_in = nc.dram_tensor("f_in", [PB, NBLK, T], f32, kind="ExternalInput").ap()
    x_in = nc.dram_tensor("x_in", [PB, NBLK, T], f32, kind="ExternalInput").ap()
    h_out = nc.dram_tensor("h_out", [PB, NBLK, T], f32, kind="ExternalOutput").ap()

    with tile.TileContext(nc) as tc:
        with (
            tc.tile_pool(name="io", bufs=2) as io_pool,
            tc.tile_pool(name="tmp", bufs=4) as tmp_pool,
        ):
            for r in range(NBLK // RB):
                bsl = slice(RB * r, RB * (r + 1))
                f_t = io_pool.tile([PB, RB, T], f32, tag="f")
                nc.sync.dma_start(out=f_t[:], in_=f_in[:, bsl, :])
                x_t = io_pool.tile([PB, RB, T], f32, tag="x")
                nc.sync.dma_start(out=x_t[:], in_=x_in[:, bsl, :])
                h_t = io_pool.tile([PB, RB, T], f32, tag="h")
                for j in range(RB):
                    a_t = tmp_pool.tile([PB, T], f32, tag="a")
                    nc.scalar.activation(
                        a_t[:], f_t[:, j, :],
                        mybir.ActivationFunctionType.Copy, bias=1.0, scale=-1.0,
                    )
                    g_t = tmp_pool.tile([PB, T], f32, tag="g")
                    nc.gpsimd.tensor_mul(g_t[:], f_t[:, j, :], x_t[:, j, :])
                    nc.vector.tensor_tensor_scan(
                        h_t[:, j, :], a_t[:], g_t[:], 0.0,
                        mybir.AluOpType.mult, mybir.AluOpType.add,
                    )
                nc.scalar.dma_start(out=h_out[:, bsl, :], in_=h_t[:])
    nc.compile()
    return nc


def _get_nc():
    if "nc" not in _cached:
        _cached["nc"] = _build()
    return _cached["nc"]


def _shard(arr):
    """[T, B, D] -> per-core [DS, B, T] (partition-major) with T reversed."""
    v = arr[::-1].transpose(2, 1, 0)  # [D, B, T] strided view, T reversed
    return [
        np.ascontiguousarray(v[DS * c : DS * (c + 1)]) for c in range(NCORES)
    ]


def _run(f, x, trace=False):
    from concourse.bass_utils import run_bass_kernel_spmd

    f = np.asarray(f, dtype=np.float32)
    x = np.asarray(x, dtype=np.float32)
    assert f.shape == (T, B, D) and x.shape == (T, B, D)

    nc = _get_nc()
    f_shards = _shard(f)
    x_shards = _shard(x)
    in_maps = [{"f_in": f_shards[c], "x_in": x_shards[c]} for c in range(NCORES)]
    res = run_bass_kernel_spmd(nc, in_maps, core_ids=list(range(NCORES)), trace=trace)

    out = np.empty((T, B, D), dtype=np.float32)
    for c in range(NCORES):
        # h_c[d, b, t_rev] -> out[t, b, DS*c + d]
        out[:, :, DS * c : DS * (c + 1)] = res.results[c]["h_out"][:, :, ::-1].transpose(2, 1, 0)
    return out.reshape(T * B, D), res


def kernel(f, x):
    return _run(f, x, trace=False)[0]


# revision 3
# speedup vs baseline: 1.2016x; 1.0538x over previous
"""Reverse-time forget-mult recurrence on 8 Trainium2 NeuronCores.

h_t = f_t*x_t + (1-f_t)*h_{t+1}, h_{T+1}=0, over [T=2048, B=16, D=1024].

Strategy: shard D across the 8 cores (128 channels each) — the recurrence is
elementwise over (B, D), sequential only in T, so no cross-core communication.
On the host, each core's shard is laid out partition-major as [D_shard=128,
B=16, T] with the T axis reversed, so each (d, b) lane's full time series is
contiguous and the device scans forward. Per 2-block step the device does one
contiguous 2 MB DMA per tensor (16 KB per-partition lines), computes
a = 1-f on the Scalar engine and g = f*x on GpSimd (keeping the Vector
engine free), and runs the whole recurrence for 128 lanes x 2048 steps in a
single hardware tensor_tensor_scan instruction (initial state 0) on Vector.
Loads issue on the Sync HWDGE ring, stores on the Scalar ring, so writes
don't head-of-line-block reads. The kernel is memory-bound: 48 MB of HBM
traffic per core.
"""

import numpy as np

T, B, D = 2048, 16, 1024
NCORES = 8
DS = D // NCORES          # 128 channels per core -> the SBUF partition dim
NBLK = B                  # 16 blocks of [128, T] per core
RB = 2                    # row-blocks per DMA (2 MB transfers)
PB = 128

_cached = {}


def _build():
    import concourse.bacc as bacc
    import concourse.mybir as mybir
    import concourse.tile as tile

    f32 = mybir.dt.float32
    nc = bacc.Bacc("TRN2", target_bir_lowering=False, debug=False, num_devices=NCORES)
    f_in = nc.dram_tensor("f_in", [PB, NBLK, T], f32, kind="ExternalInput").ap()
    x_in = nc.dram_tensor("x_in", [PB, NBLK, T], f32, kind="ExternalInput").ap()
    h_out = nc.dram_tensor("h_out", [PB, NBLK, T], f32, kind="ExternalOutput").ap()

    with tile.TileContext(nc) as tc:
        with (
            tc.tile_pool(name="io", bufs=2) as io_pool,
            tc.tile_pool(name="hp", bufs=4) as h_pool,
            tc.tile_pool(name="tmp", bufs=4) as tmp_pool,
        ):
            for r in range(NBLK // RB):
                bsl = slice(RB * r, RB * (r + 1))
                f_t = io_pool.tile([PB, RB, T], f32, tag="f")
                nc.sync.dma_start(out=f_t[:], in_=f_in[:, bsl, :])
                x_t = io_pool.tile([PB, RB, T], f32, tag="x")
                nc.sync.dma_start(out=x_t[:], in_=x_in[:, bsl, :])
                for j in range(RB):
                    a_t = tmp_pool.tile([PB, T], f32, tag="a")
                    nc.scalar.activation(
                        a_t[:], f_t[:, j, :],
                        mybir.ActivationFunctionType.Copy, bias=1.0, scale=-1.0,
                    )
                    g_t = tmp_pool.tile([PB, T], f32, tag="g")
                    nc.vector.tensor_mul(g_t[:], f_t[:, j, :], x_t[:, j, :])
                    h_t = h_pool.tile([PB, T], f32, tag="h")
                    nc.vector.tensor_tensor_scan(
                        h_t[:], a_t[:], g_t[:], 0.0,
                        mybir.AluOpType.mult, mybir.AluOpType.add,
                    )
                    nc.scalar.dma_start(out=h_out[:, RB * r + j, :], in_=h_t[:])
    nc.compile()
    return nc


def _get_nc():
    if "nc" not in _cached:
        _cached["nc"] = _build()
    return _cached["nc"]


def _shard(arr):
    """[T, B, D] -> per-core [DS, B, T] (partition-major) with T reversed."""
    v = arr[::-1].transpose(2, 1, 0)  # [D, B, T] strided view, T reversed
    return [
        np.ascontiguousarray(v[DS * c : DS * (c + 1)]) for c in range(NCORES)
    ]


def _run(f, x, trace=False):
    from concourse.bass_utils import run_bass_kernel_spmd

    f = np.asarray(f, dtype=np.float32)
    x = np.asarray(x, dtype=np.float32)
    assert f.shape == (T, B, D) and x.shape == (T, B, D)

    nc = _get_nc()
    f_shards = _shard(f)
    x_shards = _shard(x)
    in_maps = [{"f_in": f_shards[c], "x_in": x_shards[c]} for c in range(NCORES)]
    res = run_bass_kernel_spmd(nc, in_maps, core_ids=list(range(NCORES)), trace=trace)

    out = np.empty((T, B, D), dtype=np.float32)
    for c in range(NCORES):
        # h_c[d, b, t_rev] -> out[t, b, DS*c + d]
        out[:, :, DS * c : DS * (c + 1)] = res.results[c]["h_out"][:, :, ::-1].transpose(2, 1, 0)
    return out.reshape(T * B, D), res


def kernel(f, x):
    return _run(f, x, trace=False)[0]
